# revision 7
# baseline (speedup 1.0000x reference)
"""Trainium2 Bass kernel for nn_ConcentrationPredictor (v2).

RK4 integrator of dc/dt = D0 * ret(c) * lap(c) with ret = sigmoid(MLP),
N = 65536, 32 steps. 1-D domain decomposition over 8 cores; each core's
region is further split into H independent sub-slabs (each with 128-point
ghost zones) so their serial flux chains pipeline across engines.

Key differences vs v1:
- f32r matmuls (1 cycle/row for >=256-wide outputs vs 4 for fp32).
- The tall->packed fold is ONE DMA (partition-regroup access pattern on
  the packed side) instead of 16, issued on the SP queue only - the ACT
  queue no longer stalls activations behind fold DMAs.
- One activation instruction per MLP layer (strided PSUM read).
- lap(c) computed on the Vector engine early in the flux (it only needs
  c), so only ret*lap and the update are on the post-MLP critical path.
- RK accumulator is chained (acc += coef*k_s per stage) so only one
  Vector op remains after k4.
"""

import os
import sys

sys.path.insert(0, "/opt/trn_rl_repo")

import numpy as np

N_FULL = 65536
T_FULL = 33
NCORES = 8
GHOST = 128
OWN = N_FULL // NCORES  # 8192

DX = np.float32(0.04)
D0 = np.float32(0.0005 / 0.04 ** 2)   # 0.3125
DDX = np.float32(D0 * DX)             # Cauchy BC factor
BC00 = 1.0

_CACHE = {}


def _raw(inst):
    return inst.ins if hasattr(inst, "ins") else inst


def geom(H):
    sub = OWN // H                 # owned points per sub-slab
    L = sub + 2 * GHOST            # integrated length per sub-slab
    W = L // 128                   # tall width
    BW = 16 * W                    # packed width
    assert W * 128 == L
    return sub, L, W, BW


def _build(nsteps, cf2, cf4, gam, H=2, reps=1, num_devices=NCORES,
           abl=frozenset()):
    import concourse.bacc as bacc
    import concourse.tile as tile
    import concourse.mybir as mybir
    from concourse.tile_rust import add_dep_helper

    dt = mybir.dt
    AF = mybir.ActivationFunctionType
    OP = mybir.AluOpType

    sub, L, W, BW = geom(H)
    FOLD16 = os.environ.get("KERNEL_FOLD16", "1") == "1"
    FOLD16SP = os.environ.get("KERNEL_FOLD16SP", "0") == "1"
    FOLDDUMMY = os.environ.get("KERNEL_FOLDDUMMY", "0") == "1"
    FOLDDRAM = os.environ.get("KERNEL_FOLDDRAM", "1") == "1"
    HALO_SPLIT = os.environ.get("KERNEL_HALO_SPLIT", "0") == "1"
    HALO_SP = os.environ.get("KERNEL_HALO_SP", "1") == "1"

    nc = bacc.Bacc("TRN2", target_bir_lowering=False, debug=False,
                   num_devices=num_devices)

    mdt = dt.float32r

    c0s = nc.dram_tensor("c0s", [H, L], dt.float32, kind="ExternalInput").ap()
    w1s = nc.dram_tensor("w1s", [8, 120], mdt, kind="ExternalInput").ap()
    w2s = nc.dram_tensor("w2s", [120, 120], mdt, kind="ExternalInput").ap()
    w3s = nc.dram_tensor("w3s", [120, 120], mdt, kind="ExternalInput").ap()
    bqd = nc.dram_tensor("bq", [120, 2048], mdt, kind="ExternalInput").ap()
    b1d = nc.dram_tensor("b1", [120, 1], dt.float32, kind="ExternalInput").ap()
    b2d = nc.dram_tensor("b2", [120, 1], dt.float32, kind="ExternalInput").ap()
    b3d = nc.dram_tensor("b3", [120, 1], dt.float32, kind="ExternalInput").ap()
    b4d = nc.dram_tensor("b4", [128, 1], dt.float32, kind="ExternalInput").ap()
    outs = nc.dram_tensor("outs", [nsteps, H, L], dt.float32,
                          kind="ExternalOutput").ap()
    xscr = nc.dram_tensor("xscr", [H, L], dt.float32, kind="Internal").ap()

    # per-layer chunking of the packed width BW across PSUM banks
    # (each chunk 512-aligned in the psum tile; >=256 wide for full-rate f32r)
    if H == 1:
        l1_chunks = [(264 * c, 512 * c, 264) for c in range(4)]   # 4 banks
        l23_chunks = [(352 * c, 512 * c, 352) for c in range(3)]  # 3 banks
        psa_w, psb_w = 2048, 1536
        z4_in_psb = False
    else:
        cw = BW // 2
        l1_chunks = [(cw * c, 512 * c, cw) for c in range(2)]     # 2 banks
        l23_chunks = l1_chunks
        psa_w, psb_w = 1024, 1024
        z4_in_psb = True
        assert cw + W <= 512

    with tile.TileContext(nc) as tc:
        with tc.tile_pool(name="consts", bufs=1) as cp, \
             tc.tile_pool(name="state", bufs=1) as sp, \
             tc.tile_pool(name="work", bufs=3) as wp, \
             tc.tile_pool(name="psA", bufs=1, space="PSUM") as psA, \
             tc.tile_pool(name="psB", bufs=1, space="PSUM") as psB, \
             (tc.tile_pool(name="ps4", bufs=1, space="PSUM")
              if not z4_in_psb else _nullpool()) as ps4:

            w1t = cp.tile([8, 120], mdt, tag="w1t")
            nc.sync.dma_start(w1t[:], w1s[:])
            w2t = cp.tile([120, 120], mdt, tag="w2t")
            nc.sync.dma_start(w2t[:], w2s[:])
            w3t = cp.tile([120, 120], mdt, tag="w3t")
            nc.sync.dma_start(w3t[:], w3s[:])
            bqt = cp.tile([120, 2048], mdt, tag="bqt")
            nc.sync.dma_start(bqt[:], bqd[:])
            b1t = cp.tile([120, 1], dt.float32, tag="b1t")
            nc.sync.dma_start(b1t[:], b1d[:])
            b2t = cp.tile([120, 1], dt.float32, tag="b2t")
            nc.sync.dma_start(b2t[:], b2d[:])
            b3t = cp.tile([120, 1], dt.float32, tag="b3t")
            nc.sync.dma_start(b3t[:], b3d[:])
            b4t = cp.tile([128, 1], dt.float32, tag="b4t")
            nc.sync.dma_start(b4t[:], b4d[:])

            # persistent per-sub-slab state
            CE = []   # [h] -> (cea, ceb, ce2, ce3, ce4)
            XT = []   # [h] -> packed input tile
            ACC = []  # [h] -> RK accumulator
            for h in range(H):
                ces = tuple(sp.tile([128, W + 2], dt.float32, tag=f"ce{i}_{h}",
                                        name=f"ce{i}_{h}")
                            for i in range(5))
                for ce in ces:
                    nc.vector.memset(ce[:], 0.0)
                    nc.vector.memset(ce[0:1, 0:1], BC00)
                CE.append(ces)
                xt = sp.tile([8, BW], mdt, tag=f"x_{h}", name=f"x_{h}")
                nc.vector.memset(xt[:].bitcast(dt.float32), 0.0)
                XT.append(xt)
                ACC.append(sp.tile([128, W], dt.float32, tag=f"acc_{h}",
                                   name=f"acc_{h}"))

            coefs = [None, float(cf2), float(cf2), float(cf4)]

            def fold(h, ce):
                if FOLDDRAM:
                    # DRAM round-trip: two canonical DMAs
                    nc.sync.dma_start(
                        xscr[h].rearrange("(p j) -> p j", j=W),
                        ce[:, 1:W + 1])
                    return nc.sync.dma_start(
                        XT[h][0:8, :],
                        xscr[h].rearrange("(b i) -> b i", i=BW).bitcast(mdt))
                if FOLD16:
                    # v1-style: 16 canonical partition-strided DMAs split
                    # across the two HWDGE queues
                    last = None
                    for q in range(16):
                        eng = nc.sync if (q % 2 == 0 or FOLD16SP) else nc.scalar
                        srcap = (ce[0:8, 1:W + 1] if FOLDDUMMY
                                 else ce[q:128:16, 1:W + 1])
                        last = eng.dma_start(
                            XT[h][0:8, W * q:W * (q + 1)],
                            srcap.bitcast(mdt))
                    return last
                xv = XT[h][:].rearrange("b (q j) -> (b q) j", q=16)
                return nc.sync.dma_start(xv, ce[:, 1:W + 1].bitcast(mdt))

            def mm_layer(h, ps, wt, src_ap_fn, chunks, tag):
                insts = []
                for (c0, p0, cw) in chunks:
                    if "mm" in abl:
                        continue
                    insts.append(nc.tensor.matmul(
                        ps[0:120, p0:p0 + cw], wt[:], src_ap_fn(c0, cw),
                        start=True, stop=True))
                return insts

            def act_layer(ps, chunks, out, bias, func):
                if "act" in abl:
                    return None
                n = len(chunks)
                cw = chunks[0][2]
                inv = ps[0:120, 0:512 * n].rearrange(
                    "p (c k) -> p c k", k=512)[:, :, 0:cw]
                outv = out[0:120, 0:n * cw].rearrange(
                    "p (c j) -> p c j", j=cw)
                return nc.scalar.activation(outv, inv, func, bias=bias[:])

            SCHED = []
            _seq = [0]

            def at(key, fn):
                SCHED.append((key, _seq[0], fn))
                _seq[0] += 1

            ST = [dict() for _ in range(H)]

            def emit_flux_ops(h, step, s, key0):
                st = ST[h]
                base_idx = step % 2

                def p_upd():
                    base = CE[h][base_idx]
                    ce = base if s == 0 else CE[h][s + 1]
                    st["ce"] = ce
                    if s > 0:
                        nc.vector.scalar_tensor_tensor(
                            ce[:, 1:W + 1], st["kt"][:], coefs[s],
                            base[:, 1:W + 1], OP.mult, OP.add)
                    st["fold"] = fold(h, ce) if "fold" not in abl else None
                at(key0 + 0.00, p_upd)

                def p_l1():
                    zA = psA.tile([120, psa_w], dt.float32,
                                  tag=f"zA_{h}", name=f"zA_{h}")
                    mm1 = mm_layer(h, zA, w1t,
                                   lambda c0, cw: XT[h][0:8, c0:c0 + cw],
                                   l1_chunks, "L1")
                    if st["fold"] is not None and not FOLD16:
                        for mi in mm1:
                            add_dep_helper(_raw(mi), _raw(st["fold"]),
                                           reason="L1 reads fold output")
                    st["zA"] = zA
                at(key0 + 0.04, p_l1)

                def p_ghost():
                    ce = st["ce"]
                    sc = wp.tile([128, 1], dt.float32, tag=f"sc_{h}",
                                 name=f"sc_{h}")
                    nc.vector.tensor_sub(sc[:], ce[:, W - 1:W], ce[:, W:W + 1])
                    nc.vector.tensor_scalar_mul(ce[:, W + 1:W + 2], sc[:],
                                                float(DDX))
                    if "halo" not in abl:
                        if HALO_SPLIT:
                            heng = nc.sync if h == 0 else nc.scalar
                        else:
                            heng = nc.sync if HALO_SP else nc.gpsimd
                        heng.dma_start(ce[1:128, 0:1], ce[0:127, W:W + 1])
                        heng.dma_start(ce[0:127, W + 1:W + 2],
                                       ce[1:128, 1:2])
                at(key0 + 0.06, p_ghost)

                def p_tanh1():
                    h1 = wp.tile([120, BW], mdt, tag=f"h1_{h}", name=f"h1_{h}")
                    act_layer(st["zA"], l1_chunks, h1, b1t, AF.Tanh)
                    st["h1"] = h1
                at(key0 + 0.12, p_tanh1)

                def p_lap():
                    ce = st["ce"]
                    t1 = wp.tile([128, W], dt.float32, tag=f"t1_{h}",
                                 name=f"t1_{h}")
                    nc.vector.tensor_add(t1[:], ce[:, 0:W], ce[:, 2:W + 2])
                    lap = wp.tile([128, W], dt.float32, tag=f"lap_{h}",
                                  name=f"lap_{h}")
                    nc.vector.scalar_tensor_tensor(lap[:], ce[:, 1:W + 1],
                                                   -2.0, t1[:],
                                                   OP.mult, OP.add)
                    st["lap"] = lap
                at(key0 + 0.14, p_lap)

                def p_l2():
                    zB = psB.tile([128, psb_w], dt.float32, tag=f"zB_{h}",
                                  name=f"zB_{h}")
                    mm_layer(h, zB, w2t,
                             lambda c0, cw: st["h1"][:, c0:c0 + cw],
                             l23_chunks, "L2")
                    st["zB"] = zB
                at(key0 + 0.20, p_l2)

                def p_tanh2():
                    h2 = wp.tile([120, BW], mdt, tag=f"h2_{h}", name=f"h2_{h}")
                    ai = act_layer(st["zB"], l23_chunks, h2, b2t, AF.Tanh)
                    if h == 0 and step == 0 and s == 0:
                        st["anchor"] = ai
                    st["h2"] = h2
                at(key0 + 0.28, p_tanh2)

                def p_l3():
                    zC = psA.tile([120, psa_w], dt.float32,
                                  tag=f"zA_{h}", name=f"zC_{h}")
                    mm_layer(h, zC, w3t,
                             lambda c0, cw: st["h2"][:, c0:c0 + cw],
                             l23_chunks, "L3")
                    st["zC"] = zC
                at(key0 + 0.36, p_l3)

                def p_tanh3():
                    h3 = wp.tile([120, BW], mdt, tag=f"h3_{h}", name=f"h3_{h}")
                    act_layer(st["zC"], l23_chunks, h3, b3t, AF.Tanh)
                    st["h3"] = h3
                at(key0 + 0.44, p_tanh3)

                def p_l4():
                    if z4_in_psb:
                        z4 = st["zB"][0:128,
                                      l23_chunks[0][2]:l23_chunks[0][2] + W]
                    else:
                        z4t = ps4.tile([128, W], dt.float32,
                                       tag=f"z4_{h}", name=f"z4_{h}")
                        z4 = z4t[:]
                    if "mm4" not in abl and "mm" not in abl:
                        for q in range(16):
                            nc.tensor.matmul(
                                z4, bqt[:, 128 * q:128 * (q + 1)],
                                st["h3"][:, W * q:W * (q + 1)],
                                start=(q == 0), stop=(q == 15))
                    st["z4"] = z4
                at(key0 + 0.52, p_l4)

                def p_sig():
                    ret = wp.tile([128, W], dt.float32, tag=f"ret_{h}",
                                  name=f"ret_{h}")
                    nc.scalar.activation(ret[:], st["z4"], AF.Sigmoid,
                                         bias=b4t[:])
                    st["ret"] = ret
                at(key0 + 0.70, p_sig)

                def p_k():
                    kt = wp.tile([128, W], dt.float32, tag=f"kt_{h}",
                                 name=f"kt_{h}")
                    nc.vector.tensor_mul(kt[:], st["ret"][:], st["lap"][:])
                    st["kt"] = kt
                at(key0 + 0.76, p_k)

                def p_acc():
                    base = CE[h][base_idx]
                    if s == 0:
                        nc.vector.scalar_tensor_tensor(
                            ACC[h][:], st["kt"][:], float(gam),
                            base[:, 1:W + 1], OP.mult, OP.add)
                    elif s < 3:
                        nc.vector.scalar_tensor_tensor(
                            ACC[h][:], st["kt"][:], float(2 * gam),
                            ACC[h][:], OP.mult, OP.add)
                at(key0 + 0.80, p_acc)

                if s == 3:
                    def p_stepend():
                        nxt = CE[h][(step + 1) % 2]
                        nc.vector.scalar_tensor_tensor(
                            nxt[:, 1:W + 1], st["kt"][:], float(gam),
                            ACC[h][:], OP.mult, OP.add)
                        oeng = nc.sync if HALO_SP else nc.gpsimd
                        oeng.dma_start(
                            outs[step, h].rearrange("(p j) -> p j", j=W),
                            nxt[:, 1:W + 1])
                    at(key0 + 0.92, p_stepend)

            for rep in range(reps):
                rb = rep * (4 * nsteps + 2)
                for h in range(H):
                    off = h / H

                    def p_load(h=h):
                        li = nc.sync.dma_start(
                            CE[h][0][:, 1:W + 1],
                            c0s[h].rearrange("(p j) -> p j", j=W))
                        anchor = ST[0].get("anchor")
                        if h > 0 and anchor is not None:
                            add_dep_helper(_raw(li), _raw(anchor),
                                           reason="sub-slab phase offset")
                    at(rb - 1 + off if h == 0 else rb + 0.30, p_load)
                    for step in range(nsteps):
                        for s in range(4):
                            emit_flux_ops(h, step, s,
                                          rb + 4 * step + s + off)

            SCHED.sort(key=lambda t: (t[0], t[1]))
            for _, _, fn in SCHED:
                fn()

    nc.compile()
    return nc


class _nullpool:
    def __enter__(self):
        return None

    def __exit__(self, *a):
        return False


def _prep_consts(W1, b1, W2, b2, W3, b3, W4, b4, p_exp):
    scale = np.float32(10.0) ** p_exp.astype(np.float32)[0]
    w1s = np.zeros((8, 120), np.float32)
    w2s = np.zeros((120, 120), np.float32)
    w3s = np.zeros((120, 120), np.float32)
    bq = np.zeros((120, 2048), np.float32)
    w1sc = (W1.astype(np.float32)[0] * scale)  # [15]
    for r in range(8):
        w1s[r, 15 * r:15 * r + 15] = w1sc
        w2s[15 * r:15 * r + 15, 15 * r:15 * r + 15] = W2
        w3s[15 * r:15 * r + 15, 15 * r:15 * r + 15] = W3
        for q in range(16):
            bq[15 * r:15 * r + 15, 128 * q + 16 * r + q] = W4[:, 0]
    b1r = np.tile(b1.astype(np.float32), 8)[:, None]
    b2r = np.tile(b2.astype(np.float32), 8)[:, None]
    b3r = np.tile(b3.astype(np.float32), 8)[:, None]
    b4r = np.full((128, 1), np.asarray(b4, np.float32).reshape(-1)[0],
                  np.float32)
    return w1s, w2s, w3s, bq, b1r, b2r, b3r, b4r


def _slabs(c0, H=2):
    """Per-(core, sub-slab) ghost-extended slabs + owned offsets."""
    sub, L, W, BW = geom(H)
    slabs, offs = [], []
    for m in range(NCORES):
        cs, co = [], []
        for h in range(H):
            g = (m * H + h) * sub
            s0 = min(max(g - GHOST, 0), N_FULL - L)
            cs.append(c0[s0:s0 + L])
            co.append(g - s0)
        slabs.append(np.stack(cs))
        offs.append(co)
    return slabs, offs


def _make_runner(nc):
    """Persistent jitted 8-core executor for the compiled Bass program."""
    import jax
    import numpy as _np
    from jax.sharding import Mesh, PartitionSpec
    from jax.experimental.shard_map import shard_map
    import concourse.mybir as mybir
    from concourse import bass2jax

    bass2jax.install_neuronx_cc_hook()

    partition_name = (nc.partition_id_tensor.name
                      if nc.partition_id_tensor else None)
    in_names, out_names, out_avals, zero_outs = [], [], [], []
    for alloc in nc.m.functions[0].allocations:
        if not isinstance(alloc, mybir.MemoryLocationSet):
            continue
        name = alloc.memorylocations[0].name
        if alloc.kind == "ExternalInput":
            if name != partition_name:
                in_names.append(name)
        elif alloc.kind == "ExternalOutput":
            out_names.append(name)
            shape = tuple(alloc.tensor_shape)
            dtype = mybir.dt.np(alloc.dtype)
            out_avals.append(jax.core.ShapedArray(shape, dtype))
            zero_outs.append(_np.zeros(shape, dtype))
    n_params = len(in_names)
    n_outs = len(out_avals)
    all_in_names = list(in_names) + list(out_names)
    if partition_name is not None:
        all_in_names.append(partition_name)

    def _body(*args):
        operands = list(args)
        if partition_name is not None:
            operands.append(bass2jax.partition_id_tensor())
        outs = bass2jax._bass_exec_p.bind(
            *operands,
            out_avals=tuple(out_avals),
            in_names=tuple(all_in_names),
            out_names=tuple(out_names),
            lowering_input_output_aliases=(),
            sim_require_finite=True,
            sim_require_nnan=True,
            nc=nc,
        )
        return tuple(outs)

    devices = jax.devices()[:NCORES]
    mesh = Mesh(_np.asarray(devices), ("core",))
    in_specs = (PartitionSpec("core"),) * (n_params + n_outs)
    out_specs = (PartitionSpec("core"),) * n_outs
    donate = tuple(range(n_params, n_params + n_outs))
    sharded = jax.jit(
        shard_map(_body, mesh=mesh, in_specs=in_specs, out_specs=out_specs,
                  check_rep=False),
        donate_argnums=donate, keep_unused=True)

    def run(in_maps):
        per_core = [[_np.asarray(m[n]) for n in in_names] for m in in_maps]
        concat_in = [_np.concatenate([per_core[c][i] for c in range(NCORES)],
                                     axis=0) for i in range(n_params)]
        concat_zeros = [_np.zeros((NCORES * z.shape[0], *z.shape[1:]), z.dtype)
                        for z in zero_outs]
        out_arrs = sharded(*concat_in, *concat_zeros)
        out_arrs = [_np.asarray(a) for a in out_arrs]
        return [
            {name: out_arrs[i].reshape(NCORES, *out_avals[i].shape)[c]
             for i, name in enumerate(out_names)}
            for c in range(NCORES)
        ]

    return run


def kernel(c0, t, W1, b1, W2, b2, W3, b3, W4, b4, p_exp):
    c0 = np.asarray(c0, np.float32)
    t = np.asarray(t, np.float32)
    nsteps = t.shape[0] - 1
    dts = t[1:] - t[:-1]
    assert np.all(dts == dts[0]), "constant dt assumed"
    dtv = np.float32(dts[0])

    cf2 = np.float32(np.float32(0.5) * dtv * D0)
    cf4 = np.float32(dtv * D0)
    gam = np.float32((dtv / np.float32(6.0)) * D0)

    H = int(os.environ.get("KERNEL_H", "2"))
    reps = int(os.environ.get("KERNEL_REPS", "1"))
    key = (nsteps, float(dtv), H, reps,
           os.environ.get("KERNEL_FOLD16", "1"),
           os.environ.get("KERNEL_FOLDDRAM", "1"),
           os.environ.get("KERNEL_HALO_SP", "1"))
    if key not in _CACHE:
        nc = _build(nsteps, cf2, cf4, gam, H=H, reps=reps)
        _CACHE[key] = _make_runner(nc)
    run = _CACHE[key]

    w1s, w2s, w3s, bq, b1r, b2r, b3r, b4r = _prep_consts(
        np.asarray(W1), np.asarray(b1), np.asarray(W2), np.asarray(b2),
        np.asarray(W3), np.asarray(b3), np.asarray(W4), np.asarray(b4),
        np.asarray(p_exp))

    slabs, offs = _slabs(c0, H)
    in_maps = [dict(c0s=slabs[m], w1s=w1s, w2s=w2s, w3s=w3s, bq=bq,
                    b1=b1r, b2=b2r, b3=b3r, b4=b4r) for m in range(NCORES)]

    results = run(in_maps)

    sub, L, W, BW = geom(H)
    out = np.empty((nsteps + 1, N_FULL), np.float32)
    out[0] = c0
    for m in range(NCORES):
        for h in range(H):
            o = offs[m][h]
            g = (m * H + h) * sub
            out[1:, g:g + sub] = results[m]["outs"][:, h, o:o + sub]
    return out
